# revision 1
# baseline (speedup 1.0000x reference)
"""Distributed Trainium2 Bass kernel for GQA causal attention with RoPE.

Problem: B=2, S=2048, DIM=2048, 32 Q heads, 8 KV heads (GQA 4:1), hd=64,
causal, rotary embeddings, fp32 in/out.

Sharding over 8 cores: data-parallel over batch (2) x tensor-parallel over
KV-head groups (4 groups of 2 KV heads, each with its 8 Q heads).
Core c: batch b = c // 4, group g = c % 4.  Each core computes a partial
output projection; the host sums the 4 partials per batch.

v2 design (vs the 547us baseline):
- All projections in bf16 (host-cast x/wq/wo): halves DMA, enables FWL.
- Software-pipelined chunks: emission order QKV(c) | xDMA(c+1) |
  outproj(c-1) | attn(c) gives the Tile scheduler QKV(c+1)/outproj(c-1)
  matmuls as PE fillers during the ACT-bound attention phase, keeping
  the PE HAM-warm (no >3.4us gaps -> no 1.2GHz re-throttle).
- Score matmuls (K=64) for the two head-pair halves sit in disjoint PE
  row groups (base partitions 0/64) and are issued back-to-back so they
  run concurrently in the 128x128 array.
- Fine-grained causal: diagonal key tiles restrict scores/exp/mask/AV
  to the valid query columns (N=512-128r).
- Normalization per (m,h2) straight out of PSUM: reciprocal_approx_fast
  on the free denominator row, gpsimd partition_broadcast, one DVE mul.
  (The baseline's batched [128,512] DVE reciprocals cost 3.4us each.)
- QKV et order (K,V,q0..q3) so rope(kT)/v_aug complete before attention
  needs them.

Layout tricks (kept from baseline):
- head_dim permuted even-first (via Wq/Wk row permutation) so RoPE is
  32-row block ops on the DVE.
- local Q heads paired (l, l+4) per 128-row tile so the natural K tile
  provides the score stationary operand for both pair members.
- V augmented with a ones column: AV accumulates the softmax denominator
  in PSUM partition 64 for free.

Self-contained: only needs /opt/trn_rl_repo (the container's bass stack).
"""
import os
import sys

if "/opt/trn_rl_repo" not in sys.path:
    sys.path.insert(0, "/opt/trn_rl_repo")

import contextlib

import ml_dtypes
import numpy as np

import concourse.bass as bass
import concourse.tile as tile
from concourse import bacc, mybir
from concourse import bass_utils
from concourse.masks import make_identity

F32 = mybir.dt.float32
BF16 = mybir.dt.bfloat16
EXP = mybir.ActivationFunctionType.Exp

B, S, D = 2, 2048, 2048
NH, NKV, HD = 32, 8, 64
HL = 8           # local Q heads per core
KVL = 2          # local KV heads per core
EQ = HL * HD     # 512 local q features
EK = KVL * HD    # 128
EV = KVL * HD    # 128
EQKV = EQ + EK + EV  # 768
NT = S // 128    # 16 token tiles
NC = S // 512    # 4 token chunks
SCALE = 1.0 / 8.0

_CACHED_NC = None


def _rope_tile(nc, tp, ps, cosF, sinF, dst, dst_cols):
    """RoPE on a [128, 512] QKV psum tile (2 heads of 64 rows, head_dim
    permuted even-first) -> dst[0:128, dst_cols] (bf16).

    The psum tile is first evicted to bf16 SBUF on the Scalar engine
    (idle during QKV phases); the rope muls/adds then run all-bf16
    all-SBUF on the DVE, which enables its 4x perf mode.

    For each head block at base o in {0, 64}:
      out[o:o+32]    = p[o:o+32]*cos    - p[o+32:o+64]*sin
      out[o+32:o+64] = p[o+32:o+64]*cos + p[o:o+32]*sin
    t1 = swap(p) * sinF (sinF rows: -sin,+sin), t2 = p * cosF,
    out = t2 + t1.
    """
    qe = tp.tile([128, 512], BF16, tag="rope_src")
    nc.vector.tensor_copy(qe[:], ps[:])
    # partition swap via 1-input copies (all-SBUF tensor_tensor requires
    # equal base partitions, so the swap cannot fold into the mul)
    qs = tp.tile([128, 512], BF16, tag="rope_sw")
    for o in (0, 64):
        nc.vector.tensor_copy(qs[o:o + 32, :], qe[o + 32:o + 64, :])
        nc.vector.tensor_copy(qs[o + 32:o + 64, :], qe[o:o + 32, :])
    t1 = tp.tile([128, 512], BF16, tag="rope_t1")
    nc.vector.tensor_mul(t1[:], qs[:], sinF[:])
    t2 = tp.tile([128, 512], BF16, tag="rope_t2")
    nc.vector.tensor_mul(t2[:], qe[:], cosF[:])
    nc.vector.tensor_add(dst[:, dst_cols], t2[:], t1[:])


def build():
    nc = bacc.Bacc("TRN2", target_bir_lowering=False, debug=False)
    # x is host-transposed and bf16: [D, S]
    x_d = nc.dram_tensor("x", [D, S], BF16, kind="ExternalInput").ap()
    wq_d = nc.dram_tensor("wq", [D, EQKV], BF16, kind="ExternalInput").ap()
    wo_d = nc.dram_tensor("wo", [EQ, D], BF16, kind="ExternalInput").ap()
    # rope rows: 0:128 = cos x4, 128:256 = [-sin, +sin] x2
    rope_d = nc.dram_tensor("rope", [256, S], BF16, kind="ExternalInput").ap()
    out_d = nc.dram_tensor("out", [S, D], BF16, kind="ExternalOutput").ap()

    # qT[0],qT[1] first (the m01 pass reads old-chunk kT/v for its first
    # key tiles, so it only needs its q tiles to start), then kT, V, q2/q3.
    ET_ORDER = (0, 1, 4, 5, 2, 3)

    with tile.TileContext(nc) as tc:
        ctx = contextlib.ExitStack()
        with ctx:
            const = ctx.enter_context(tc.tile_pool(name="const", bufs=1))
            persist = ctx.enter_context(tc.tile_pool(name="persist", bufs=1))
            xtp = ctx.enter_context(tc.tile_pool(name="xt", bufs=2))
            ropep = ctx.enter_context(tc.tile_pool(name="ropep", bufs=2))
            vtp = ctx.enter_context(tc.tile_pool(name="vtp", bufs=2))
            pbp = ctx.enter_context(tc.tile_pool(name="pbp", bufs=8))
            rcp = ctx.enter_context(tc.tile_pool(name="rcp", bufs=4))
            rbp = ctx.enter_context(tc.tile_pool(name="rbp", bufs=4))
            y_pool = ctx.enter_context(tc.tile_pool(name="yp", bufs=4))
            # PSUM: aux (QKV groups + outproj psy + v transposes) 2 banks,
            # scores pair tile [128,2,512] 2 banks, po accumulators 4 = 8.
            ps_aux = ctx.enter_context(
                tc.tile_pool(name="psaux", bufs=2, space="PSUM"))
            ps_ss = ctx.enter_context(
                tc.tile_pool(name="psss", bufs=2, space="PSUM"))
            ps_po = ctx.enter_context(
                tc.tile_pool(name="pspo", bufs=4, space="PSUM"))

            # ---- constants / weights (DMA order matters: x chunk0 + wq
            # interleaved so QKV(0) can start almost immediately) ----
            ident = const.tile([128, 128], BF16)
            make_identity(nc, ident[:])
            cosF = const.tile([128, S], BF16)
            nc.sync.dma_start(cosF[:], rope_d[0:128, :])
            sinF = const.tile([128, S], BF16)
            nc.sync.dma_start(sinF[:], rope_d[128:256, :])

            # Bulk 3D-strided DMAs: each sync.dma_start costs ~0.6us of
            # sync-engine issue time, so merge the per-dt transfers.
            xr = x_d.rearrange("(dt p) s -> p dt s", p=128)
            wqr = wq_d.rearrange("(dt p) e -> p dt e", p=128)
            wor = wo_d.rearrange("(dt p) e -> p dt e", p=128)
            # wq arrives in ET_ORDER-matched waves: q0 cols (0:128) with the
            # chunk-0 x tile first, then K/V cols, then q1..q3 cols.
            wq_sb = const.tile([128, 16, EQKV], BF16, name="wq_sb")
            x_tiles = [None] * NC
            xt0 = xtp.tile([128, 16, 512], BF16, tag="xt", name="xT_0")
            for g in range(4):
                sl = slice(4 * g, 4 * g + 4)
                nc.sync.dma_start(wq_sb[:, sl, 0:128], wqr[:, sl, 0:128])
                nc.sync.dma_start(xt0[:, sl, :], xr[:, sl, 0:512])
            x_tiles[0] = xt0
            nc.sync.dma_start(wq_sb[:, :, 512:768], wqr[:, :, 512:768])
            nc.sync.dma_start(wq_sb[:, :, 128:512], wqr[:, :, 128:512])
            wo_sb = const.tile([128, 4, D], BF16, name="wo_sb")
            nc.sync.dma_start(wo_sb[:], wor[:])
            # causal-mask constants: negI = -1e6 * I, tri[k, j] = 1 if k > j.
            # Masking is applied on the PE: ss += negI.T @ tri adds -1e6 to
            # key>query positions before the exp (avoids gpsimd on the
            # attention critical path).
            negI = const.tile([128, 128], BF16, name="negI")
            nc.scalar.activation(negI[:], ident[:],
                                 mybir.ActivationFunctionType.Copy,
                                 scale=-1.0e6)
            tri = const.tile([128, 512], BF16, name="tri")
            ones_t = const.tile([128, 512], BF16, name="ones_t")
            nc.gpsimd.memset(ones_t[:], 1.0)
            nc.gpsimd.affine_select(
                out=tri[:], in_=ones_t[:],
                compare_op=mybir.AluOpType.is_ge, fill=0.0,
                base=-1, channel_multiplier=1, pattern=[[-1, 512]])

            # ---- persistent activation buffers ----
            qT = [persist.tile([128, S], BF16, tag=f"qT{i}", name=f"qT{i}")
                  for i in range(4)]
            kT = persist.tile([128, S], BF16, name="kT")
            v_aug = [persist.tile([128, 130], BF16, tag=f"vaug{i}",
                                  name=f"vaug{i}") for i in range(NT)]
            aoT = [persist.tile([128, S], BF16, tag=f"aoT{i}", name=f"aoT{i}")
                   for i in range(4)]
            # ones columns of v_aug are written once
            for it in range(NT):
                nc.gpsimd.memset(v_aug[it][:, 64:65], 1.0)
                nc.gpsimd.memset(v_aug[it][:, 129:130], 1.0)

            def emit_qkv(c):
                tcol = slice(512 * c, 512 * (c + 1))
                cos_c = cosF[:, tcol]
                sin_c = sinF[:, tcol]
                xt = x_tiles[c]
                for et in ET_ORDER:
                    ps = ps_aux.tile([128, 512], F32, tag="aux")
                    for dt in range(16):
                        nc.tensor.matmul(
                            ps[:], wq_sb[:, dt, 128 * et:128 * (et + 1)],
                            xt[:, dt, :], start=(dt == 0), stop=(dt == 15))
                    if et < 4:
                        _rope_tile(nc, ropep, ps, cos_c, sin_c, qT[et], tcol)
                    elif et == 4:
                        _rope_tile(nc, ropep, ps, cos_c, sin_c, kT, tcol)
                    else:
                        # vT [e_v, t] -> evict bf16, PE-transpose to v_aug
                        vt = vtp.tile([128, 512], BF16, tag="vt")
                        nc.vector.tensor_copy(vt[:], ps[:])
                        pt = ps_aux.tile([128, 512], BF16, tag="aux")
                        for tt in range(4):
                            nc.tensor.transpose(
                                pt[:, 128 * tt:128 * (tt + 1)],
                                vt[:, 128 * tt:128 * (tt + 1)], ident[:])
                        for tt in range(4):
                            it = 4 * c + tt
                            sl = slice(128 * tt, 128 * tt + 64)
                            nc.vector.tensor_copy(v_aug[it][:, 0:64],
                                                  pt[:, sl])
                            sl = slice(128 * tt + 64, 128 * (tt + 1))
                            nc.vector.tensor_copy(v_aug[it][:, 65:129],
                                                  pt[:, sl])

            def emit_xdma(c):
                xt = xtp.tile([128, 16, 512], BF16, tag="xt", name=f"xT_{c}")
                for g in range(4):
                    nc.sync.dma_start(
                        xt[:, 4 * g:4 * g + 4, :],
                        xr[:, 4 * g:4 * g + 4, 512 * c:512 * (c + 1)])
                x_tiles[c] = xt

            def emit_outproj(c):
                for tt in range(4):
                    trow = slice(512 * c + 128 * tt, 512 * c + 128 * (tt + 1))
                    ysb = y_pool.tile([128, D], BF16, tag="ysb")
                    for ec in range(4):
                        psy = ps_aux.tile([128, 512], F32, tag="aux")
                        for dt in range(4):
                            nc.tensor.matmul(
                                psy[:], aoT[dt][:, trow],
                                wo_sb[:, dt, 512 * ec:512 * (ec + 1)],
                                start=(dt == 0), stop=(dt == 3))
                        nc.vector.tensor_copy(
                            ysb[:, 512 * ec:512 * (ec + 1)], psy[:])
                    nc.sync.dma_start(out_d[trow, :], ysb[:])

            def emit_attn(c):
                # kt-outer over m-pairs: consecutive matmuls share their
                # stationary operand (kT slice / v_aug slice / negI), so the
                # PE amortizes LDWEIGHTS across the pair instead of paying
                # load+stream serially on every matmul.
                n_tk = 4 * (c + 1)
                tcol = slice(512 * c, 512 * (c + 1))
                for mp in range(2):
                    ms = (2 * mp, 2 * mp + 1)
                    po = {m: [ps_po.tile([65, 512], F32, tag="po",
                                         name=f"po_{c}_{m}_{i}")
                              for i in range(2)] for m in ms}
                    for kt in range(n_tk):
                        r = kt - 4 * c
                        # valid query cols within the chunk for this key tile
                        lo = 128 * r if r > 0 else 0
                        cols = slice(lo, 512)
                        qcols = slice(512 * c + lo, 512 * (c + 1))
                        diag = r >= 0
                        pv = {m: [None, None] for m in ms}
                        for h2 in range(2):
                            o = 64 * h2
                            ss = {m: ps_ss.tile([128, 512], F32, tag="ss",
                                                name=f"ss_{c}_{m}_{kt}_{h2}")
                                  for m in ms}
                            for m in ms:
                                nc.tensor.matmul(
                                    ss[m][:, cols],
                                    kT[o:o + 64, 128 * kt:128 * (kt + 1)],
                                    qT[m][o:o + 64, qcols],
                                    start=True, stop=not diag)
                            if diag:
                                # add -1e6 to key>query positions via the
                                # PE; tri is zero beyond col 127, so only
                                # the 128-wide diagonal square is needed.
                                for m in ms:
                                    nc.tensor.matmul(
                                        ss[m][:, lo:lo + 128], negI[:],
                                        tri[:, 0:128],
                                        start=False, stop=True)
                            for m in ms:
                                pbf = pbp.tile([128, 512], BF16, tag="pbf")
                                nc.scalar.activation(pbf[:, cols],
                                                     ss[m][:, cols],
                                                     EXP, scale=SCALE)
                                pv[m][h2] = pbf
                        for h2 in range(2):
                            for m in ms:
                                nc.tensor.matmul(
                                    po[m][h2][:, cols],
                                    v_aug[kt][:, 65 * h2:65 * h2 + 65],
                                    pv[m][h2][:, cols],
                                    start=(kt == 0), stop=(kt == n_tk - 1))
                    # normalize per (m, h2): denominator row PSUM->SBUF,
                    # fast reciprocal, gpsimd broadcast, one DVE mul.
                    for m in ms:
                        for h2 in range(2):
                            dn = rcp.tile([1, 512], F32, tag="dn")
                            nc.vector.tensor_copy(dn[:], po[m][h2][64:65, :])
                            rc = rcp.tile([1, 512], F32, tag="rc")
                            nc.vector.reciprocal_approx_fast(rc[:], dn[:])
                            rb = rbp.tile([64, 512], F32, tag="rb")
                            nc.gpsimd.partition_broadcast(rb[:], rc[:])
                            nc.vector.tensor_mul(
                                aoT[m][64 * h2:64 * h2 + 64, tcol],
                                po[m][h2][0:64, :], rb[:])

            for c in range(NC):
                emit_qkv(c)
                if c + 1 < NC:
                    emit_xdma(c + 1)
                if c >= 1:
                    emit_outproj(c - 1)
                emit_attn(c)
            emit_outproj(NC - 1)

    nc.compile()
    return nc


# Local Q heads are processed in pairs (l, l+4): pair tile m holds head l
# at rows 0:64 (kv j=0) and head l+4 at rows 64:128 (kv j=1).
HEAD_ORDER = [0, 4, 1, 5, 2, 6, 3, 7]


def _prep_inputs(x, freqs_cis, wqkv, wo):
    """Host-side sharding: returns list of 8 in_maps."""
    bf16 = ml_dtypes.bfloat16
    perm = np.concatenate([np.arange(0, HD, 2), np.arange(1, HD, 2)])
    cos = np.ascontiguousarray(freqs_cis[:, :, 0].T.astype(np.float32))  # [32,S]
    sin = np.ascontiguousarray(freqs_cis[:, :, 1].T.astype(np.float32))
    rope = np.ascontiguousarray(
        np.concatenate([cos, cos, cos, cos, -sin, sin, -sin, sin],
                       axis=0).astype(bf16))  # [256,S]
    xT_by_b = [np.ascontiguousarray(x[b].T.astype(bf16)) for b in range(B)]
    in_maps = []
    for c in range(8):
        b, g = c // 4, c % 4
        # [HL, HD, D] with head_dim even-first permutation + head pairing
        wq_rows = wqkv[EQ * g:EQ * (g + 1)].reshape(HL, HD, D)[:, perm, :]
        wq_rows = wq_rows[HEAD_ORDER].reshape(EQ, D)
        wk_rows = wqkv[D + EK * g:D + EK * (g + 1)].reshape(
            KVL, HD, D)[:, perm, :].reshape(EK, D)
        wv_rows = wqkv[D + NKV * HD + EV * g:D + NKV * HD + EV * (g + 1)]
        wq_cat = np.concatenate([wq_rows, wk_rows, wv_rows], axis=0)
        # woT rows reordered to the paired-head d-block layout
        woT = wo[:, EQ * g:EQ * (g + 1)].T.reshape(HL, HD, D)
        woT = woT[HEAD_ORDER].reshape(EQ, D)
        in_maps.append({
            "x": xT_by_b[b],
            "wq": np.ascontiguousarray(wq_cat.T.astype(bf16)),
            "wo": np.ascontiguousarray(woT.astype(bf16)),
            "rope": rope,
        })
    return in_maps


def _get_nc():
    global _CACHED_NC
    if _CACHED_NC is None:
        _CACHED_NC = build()
    return _CACHED_NC


def kernel(x, freqs_cis, wqkv, wo, _trace=False, _trace_kwargs=None):
    nc = _get_nc()
    in_maps = _prep_inputs(x, freqs_cis, wqkv, wo)
    res = bass_utils.run_bass_kernel_spmd(
        nc, in_maps, core_ids=list(range(8)), trace=_trace,
        **(_trace_kwargs or {}))
    outs = [np.asarray(res.results[c]["out"], dtype=np.float32)
            for c in range(8)]
    y = np.stack([
        outs[0] + outs[1] + outs[2] + outs[3],
        outs[4] + outs[5] + outs[6] + outs[7],
    ]).astype(np.float32)
    kernel.last_results = res
    return y

